# revision 2
# baseline (speedup 1.0000x reference)
"""Trainium2 Bass kernel for nn_CINTransform: out[b,h,f] = sum_ij w[h,i,j]*in1[b,i,f]*in2[b,j,f].

Sharding: data-parallel over batch B=2048 across 8 NeuronCores (256 batches per
core); the small weight is replicated (pre-arranged host-side as [i,(h,j)]).

Per-core algorithm (per batch b):
  phase A (PE):  L[f,(h,j)] = sum_i in1[b,i,f] * Warr[i,(h,j)]   (bf16 matmul, fp32 psum)
  phase B (DVE): tmp[f,h,j] = L[f,(h,j)] * in2t[b,f,j]  (h broadcast)
                 out[f,b,h] = sum_j tmp[f,h,j]
Output is produced in [f, b, h] layout on device; the host transposes back.
"""

import numpy as np
import ml_dtypes

import concourse.bacc as bacc
import concourse.bass as bass
import concourse.mybir as mybir
import concourse.tile as tile
from concourse import bass_utils


def _install_ntff_shim():
    """The image's antenv stub lacks axon_hooks; inject one so trace=True works."""
    import sys
    import types

    if "antenv.axon_hooks" in sys.modules:
        return
    try:
        from trn_agent_boot.trn_boot import _ntff_profile_via_ctypes

        hook = _ntff_profile_via_ctypes("/opt/axon/libaxon_pjrt.so")
    except Exception:
        hook = None
    mod = types.ModuleType("antenv.axon_hooks")
    mod.get_axon_ntff_profile_hook = lambda: hook
    mod.set_axon_ntff_profile_hook = lambda h: None
    sys.modules["antenv.axon_hooks"] = mod


_install_ntff_shim()

B, H, D1, D2, F = 2048, 16, 64, 64, 128
NCORES = 8
BLOC = B // NCORES  # 256
HJ = H * D2  # 1024
OB = 32  # batches per output DMA

_nc_cache = {}


def _build_nc():
    if "nc" in _nc_cache:
        return _nc_cache["nc"]
    nc = bacc.Bacc("TRN2", target_bir_lowering=False)
    bf16 = mybir.dt.bfloat16
    f32 = mybir.dt.float32

    in1_d = nc.dram_tensor("in1", [BLOC, D1, F], bf16, kind="ExternalInput")
    in2t_d = nc.dram_tensor("in2t", [BLOC, F, D2], bf16, kind="ExternalInput")
    w_d = nc.dram_tensor("w", [D1, HJ], bf16, kind="ExternalInput")
    out_d = nc.dram_tensor("out", [F, BLOC, H], f32, kind="ExternalOutput")

    with tile.TileContext(nc) as tc:
        with (
            tc.tile_pool(name="wpool", bufs=1) as wpool,
            tc.tile_pool(name="io", bufs=6) as io,
            tc.tile_pool(name="tmp", bufs=3) as tmppool,
            tc.tile_pool(name="outp", bufs=2) as outp,
            tc.tile_pool(name="ps", bufs=2, space="PSUM") as ps,
        ):
            w_sb = wpool.tile([D1, HJ], bf16)
            nc.sync.dma_start(out=w_sb[:], in_=w_d[:])

            for bo in range(BLOC // OB):
                out_sb = outp.tile([F, OB, H], f32)
                for bi in range(OB):
                    b = bo * OB + bi
                    in1_sb = io.tile([D1, F], bf16, tag="in1")
                    nc.sync.dma_start(out=in1_sb[:], in_=in1_d[b])
                    in2t_sb = io.tile([F, D2], bf16, tag="in2t")
                    nc.sync.dma_start(out=in2t_sb[:], in_=in2t_d[b])

                    psum_l = ps.tile([F, HJ], f32)
                    nc.tensor.matmul(
                        psum_l[:, 0:512], in1_sb[:], w_sb[:, 0:512]
                    )
                    nc.tensor.matmul(
                        psum_l[:, 512:1024], in1_sb[:], w_sb[:, 512:1024]
                    )

                    tmp = tmppool.tile([F, H, D2], bf16)
                    in2_bc = in2t_sb[:].unsqueeze(1).broadcast_to([F, H, D2])
                    nc.vector.tensor_mul(
                        out=tmp[:],
                        in0=psum_l[:].rearrange("f (h j) -> f h j", h=H),
                        in1=in2_bc,
                    )
                    nc.vector.reduce_sum(
                        out=out_sb[:, bi, :], in_=tmp[:], axis=mybir.AxisListType.X
                    )
                nc.sync.dma_start(
                    out=out_d[:, bo * OB : (bo + 1) * OB, :], in_=out_sb[:]
                )
    nc.compile()
    _nc_cache["nc"] = nc
    return nc


def _prep_inputs(input1, input2, weight):
    """Host-side layout prep: cast to bf16, shard over B, transpose in2, arrange w."""
    bf = ml_dtypes.bfloat16
    in1 = np.ascontiguousarray(input1.astype(bf))  # [B, D1, F]
    in2t = np.ascontiguousarray(input2.astype(bf).transpose(0, 2, 1))  # [B, F, D2]
    warr = np.ascontiguousarray(
        weight.astype(bf).transpose(1, 0, 2).reshape(D1, HJ)
    )  # [i, (h,j)]
    in_maps = []
    for c in range(NCORES):
        sl = slice(c * BLOC, (c + 1) * BLOC)
        in_maps.append(
            {
                "in1": np.ascontiguousarray(in1[sl]),
                "in2t": np.ascontiguousarray(in2t[sl]),
                "w": warr,
            }
        )
    return in_maps


def kernel(input1, input2, weight, _trace=False):
    nc = _build_nc()
    in_maps = _prep_inputs(input1, input2, weight)
    res = bass_utils.run_bass_kernel_spmd(
        nc, in_maps, core_ids=list(range(NCORES)), trace=_trace
    )
    outs = []
    for c in range(NCORES):
        o = res.results[c]["out"]  # [F, BLOC, H]
        outs.append(np.ascontiguousarray(o.transpose(1, 2, 0)))  # [BLOC, H, F]
    full = np.concatenate(outs, axis=0).astype(np.float32)  # [B, H, F]
    if _trace:
        kernel.last_results = res
    return full
